# revision 10
# baseline (speedup 1.0000x reference)
"""Trainium2 Bass kernel for single-head cross-modal attention.

Problem: B=8, S=2048, D=1024 (fp32 inputs)
    q = image_emb @ Wq.T + bq
    k = text_emb  @ Wk.T + bk
    v = text_emb  @ Wv.T + bv
    out = softmax(q @ k.T / sqrt(D)) @ v
Sharding: data-parallel over batch — core b handles batch element b.

Key algebraic restructure (kills the on-device K projection):
    q k^T = Xi (Wq^T Wk) Xt^T + (per-query-row constants) + 1 (bq^T Wk) Xt^T
Softmax is row-shift invariant, so the row-constant terms drop. With
host-precomputed M = Wq^T Wk and c = bq @ Wk:
    scores ~ (Xi M + 1 c^T) Xt^T
The Q projection becomes A = Xi M + c (same cost, bias reused) and K^T
is just Xt^T — DMA'd once into SBUF and reused both as the scores
stationary and as the V-projection stationary.

Startup (trace-driven): nothing can land in SBUF before ~12.5us (code
load ~3us, engine barrier, first DMA issue ~7.2us, DMA pipe latency).
Meanwhile the PE HAM clock gate is cold (1.2 GHz) and only goes warm
after ~3.4us of gap-free matmul activity. So the kernel front-loads a
dependency-free bf16 warm-up chain (memset tile, accumulated into a
dead PSUM bank) sized to cover the DMA latency window: by the time the
first A-phase operand arrives, the PE is already at 2.4 GHz and the A
phase streams gap-free. The bv broadcast is host-prepared and DMA'd
(the old on-chip fp32 ones@bv matmul broadcast ran in 2-pass fp32 mode
and wasted ~4us of the warm-up window).

DMA strategy: every dma_start costs ~0.7us on the issuing sequencer and
the two HWDGE rings share the 16 SDMA engines (~0.43 MB/us aggregate,
global FIFO-ish in issue order). Descriptors are ordered by first use:
first A-chain operands (m row 0, xi chunk 0) lead both rings, the rest
of m/xi follow interleaved, V/attention bulk (wv, kt) and the
bv broadcast ride the tails.

Per-core dataflow (matmuls bf16, fp32 PSUM):
  - AT[d',q] per 512-col chunk: stationary M[d,d'_tile], moving XiT.
  - scoresT[k,q] = kt_tile.T @ AT -> exp -> stationary of P@V: the
    2048x2048 probability matrix is never transposed on chip.
  - no-max softmax (scores ~ N(0,1)); V carries an appended ones
    column, and P@V runs as three chains (512/256/257 cols) so the
    softmax denominators fall out of the third chain's last column and
    every matmul's ldweights hides under a >=256-row stream; final
    normalize fused with the bv add in one DVE op per chunk.
  - last q_tile runs denominator-chain-first (257+256+256 interleaved,
    then a lone 256-col chain) so only one small drain+store trails the
    final matmul.
  - all pools live in one scope (no mid-kernel scope-exit barrier);
    single 8-bank PSUM ring shared by every phase.
"""

import sys
import os

for _p in ("/opt/trn_rl_repo", "/root/.axon_site/_ro/trn_rl_repo"):
    if os.path.isdir(_p) and _p not in sys.path:
        sys.path.insert(0, _p)

import numpy as np
import ml_dtypes

import concourse.bass as bass
import concourse.mybir as mybir
import concourse.tile as tile
from concourse import bacc
from concourse.bass_utils import run_bass_kernel_spmd

BF16 = mybir.dt.bfloat16
F32 = mybir.dt.float32
AF = mybir.ActivationFunctionType
ALU = mybir.AluOpType

B, S, D = 8, 2048, 1024
P = 128
ND = D // P          # 8  d tiles
NS = S // P          # 16 s tiles
QC = 512             # q chunk width (matmul free dim / PSUM bank)
NQC = S // QC        # 4
EC = 512             # e chunk width for V / output
SCALE = 1.0 / float(np.sqrt(D))
NWARM = 17           # dummy warm-up matmuls (~5.8us cold->warm)

_CACHE = {}


def _build_nc():
    nc = bacc.Bacc("TRN2", target_bir_lowering=False, debug=False, num_devices=8)

    # all bulk tensors are pre-relayouted on host to SBUF tile order
    xi_d = nc.dram_tensor("xi", [P, NQC, ND * QC], BF16, kind="ExternalInput").ap()
    xt_d = nc.dram_tensor("xt", [P, ND, S], BF16, kind="ExternalInput").ap()
    m_d = nc.dram_tensor("m", [P, ND, ND * P], BF16, kind="ExternalInput").ap()
    wvt_d = nc.dram_tensor("wvt", [P, ND, D], BF16, kind="ExternalInput").ap()
    ca_d = nc.dram_tensor("ca", [P, ND], F32, kind="ExternalInput").ap()
    bv2_d = nc.dram_tensor("bv2", [P, D], F32, kind="ExternalInput").ap()
    out_d = nc.dram_tensor("out", [S, D], F32, kind="ExternalOutput").ap()

    with tile.TileContext(nc) as tc:
        _emit(nc, tc, xi_d, xt_d, m_d, wvt_d, ca_d, bv2_d, out_d)
    nc.compile()
    return nc


def _emit(nc, tc, xi_d, xt_d, m_d, wvt_d, ca_d, bv2_d, out_d):
    NH = QC // P  # 4 q_tiles per chunk
    with (
        tc.tile_pool(name="const", bufs=1) as pc,
        tc.tile_pool(name="qkv", bufs=1) as pqkv,
    ):
        # persistent activations
        at = pqkv.tile([P, ND, S], BF16, name="at", tag="at")    # AT[d',q]
        kt = pqkv.tile([P, ND, S], BF16, name="kt", tag="kt")    # XtT[d',k]
        # V[s, e] with a ones column appended at e=D: the P@V row sums
        # (softmax denominators) fall out of the last PV chain, so no
        # 1-column rowsum matmuls (whose ldweights never hide) are needed.
        v = pqkv.tile([P, NS, D + 1], BF16, name="v", tag="v")

        # constants
        bias_a = pc.tile([P, ND], F32, name="bias_a", tag="bias_a")
        bv_bcast = pc.tile([P, D], F32, name="bv_bcast", tag="bv_bcast")
        warm = pc.tile([P, QC], BF16, name="warm", tag="warm")

        with (
            tc.tile_pool(name="w", bufs=1) as pw,
            tc.tile_pool(name="xs", bufs=1) as pxs,
            tc.tile_pool(name="psP", bufs=8, space="PSUM") as psP,
            tc.tile_pool(name="et", bufs=2) as pet,
            tc.tile_pool(name="outp", bufs=1) as pout,
            tc.tile_pool(name="stat", bufs=4) as pstat,
        ):
            psST = psAV = psP
            # m_sb[:, et, d*P:(d+1)*P] = M[d-block, et-block] (et-major!)
            m_sb = pw.tile([P, ND, ND * P], BF16, name="m_sb", tag="m_sb")
            wv_sb = pw.tile([P, ND, D], BF16, name="wv_sb", tag="wv_sb")
            # xc[:, qc, d*QC:(d+1)*QC] = XiT[d-block, qc-chunk]
            xc = pxs.tile([P, NQC, ND * QC], BF16, name="xc", tag="xs")

            # --- bulk DMAs, ordered by first use. Two hard constraints
            # (trace-measured):
            #  * the rings share ONE pool of 8 completion semaphores,
            #    allocated in global dma_start program order; descriptor
            #    9+ REUSES a semaphore and its issue blocks until the
            #    previous user completes. So the 8 A-phase-critical
            #    descriptors go first, and each reuser is paired with an
            #    early-completing predecessor.
            #  * aggregate delivery is ~0.4 MB/us from ~9.5us, roughly
            #    in issue order; a warm A chain consumes 0.22 MB/us.
            # NOTE: gpsimd triggers lower to qPoolDynamic (slow SW
            # completions) — avoid; everything rides sync/scalar HWDGE.
            # Delivery is ~0.4 MB/us aggregate in issue order, and is
            # packet-rate-bound: m descriptors spanning >=2 et-rows move
            # as 4KB packets (per-partition-contiguous), single rows
            # only 2KB — so m ships as 2/2/4-row chunks. First A-chain
            # bytes lead; everything is just-in-time for a fully warm,
            # gap-free A phase from ~12.4us.
            # (xi/m slices below keep >=4KB per-partition-contiguous
            # runs; finer slicing halves the packet rate and loses more
            # than the granularity wins.)
            nc.sync.dma_start(m_sb[:, 0:2, :], m_d[:, 0:2, :])    # 512KB
            nc.scalar.dma_start(xc[:, 0, 0:4 * QC], xi_d[:, 0, 0:4 * QC])
            nc.sync.dma_start(xc[:, 0, 4 * QC:], xi_d[:, 0, 4 * QC:])
            nc.scalar.dma_start(m_sb[:, 2:4, :], m_d[:, 2:4, :])  # 512KB
            nc.sync.dma_start(m_sb[:, 4:8, :], m_d[:, 4:8, :])    # 1MB
            nc.scalar.dma_start(bias_a[:], ca_d[:])               # 4KB
            nc.sync.dma_start(xc[:, 1, :], xi_d[:, 1, :])         # 1MB
            nc.scalar.dma_start(xc[:, 2, :], xi_d[:, 2, :])
            # -- semaphore pool (8) exhausted: each of the following
            # reuses the semaphore of the descriptor 8 positions earlier
            # and issues only once that one completes — paired so every
            # issue-wait is satisfied before the data is needed.
            nc.sync.dma_start(xc[:, 3, :], xi_d[:, 3, :])
            nc.scalar.dma_start(wv_sb[:], wvt_d[:])               # 2MB
            nc.sync.dma_start(kt[:, 0:4, :], xt_d[:, 0:4, :])
            nc.scalar.dma_start(kt[:, 4:8, :], xt_d[:, 4:8, :])
            # bv broadcast: first consumer is the PV phase (~165us)
            nc.sync.dma_start(bv_bcast[:], bv2_d[:])              # 512KB
            nc.vector.memset(warm[:], 1.0)
            nc.vector.memset(v[:, :, D:D + 1], 1.0)

            # --- dependency-free warm-up chain: keeps the PE busy from
            # ~7.5us (right after the framework preamble) through the
            # ~12.5us DMA pipe latency, flipping the HAM clock gate to
            # 2.4 GHz before the first real chain. Accumulates ones into
            # a dead PSUM bank; never read.
            ps_w = psP.tile([P, QC], F32, name="ps_w", tag="ps")
            for i in range(NWARM):
                nc.tensor.matmul(ps_w[:], warm[:, 0:P], warm[:],
                                 start=(i == 0), stop=(i == NWARM - 1))

            # --- AT[d'_t, qc] = sum_d M[d, d'_t].T @ XiT[d, qc]  (+ c) ---
            # qc 0 runs during the DMA supply transient: its first four
            # et chains are emitted as interleaved d-half blocks, so all
            # work needing only the earliest descriptors (m rows 0-1,
            # xi chunk 0 first half) runs first and a late xc0b/m23
            # stalls the PE briefly instead of ~3us (which would also
            # re-throttle the HAM clock gate).
            ps_h = [psP.tile([P, QC], F32, name=f"psh{et}", tag="ps")
                    for et in range(4)]
            for half in range(2):
                for ep in range(2):
                    for d in range(half * 4, half * 4 + 4):
                        for et in (2 * ep, 2 * ep + 1):
                            nc.tensor.matmul(
                                ps_h[et][:], m_sb[:, et, d * P:(d + 1) * P],
                                xc[:, 0, d * QC:(d + 1) * QC],
                                start=(d == 0), stop=(d == ND - 1))
            for et in range(4):
                nc.vector.tensor_scalar_add(
                    at[:, et, 0:QC], ps_h[et][:], bias_a[:, et:et + 1])
            for qc in range(NQC):
                for et in range(4 if qc == 0 else 0, ND):
                    ps = psP.tile([P, QC], F32, name="ps", tag="ps")
                    for d in range(ND):
                        nc.tensor.matmul(
                            ps[:], m_sb[:, et, d * P:(d + 1) * P],
                            xc[:, qc, d * QC:(d + 1) * QC],
                            start=(d == 0), stop=(d == ND - 1))
                    nc.vector.tensor_scalar_add(
                        at[:, et, qc * QC:(qc + 1) * QC], ps[:],
                        bias_a[:, et:et + 1])

            # --- V[s_t, e] = sum_d XtT[d, s_t].T @ Wv^T[d, e] ---
            for vc in range(NQC):
                for si in range(NH):
                    st = vc * NH + si
                    ps0 = psP.tile([P, EC], F32, name="ps0", tag="ps")
                    ps1 = psP.tile([P, EC], F32, name="ps1", tag="ps")
                    for d in range(ND):
                        lhs = kt[:, d, st * P:(st + 1) * P]
                        nc.tensor.matmul(ps0[:], lhs, wv_sb[:, d, 0:EC],
                                         start=(d == 0), stop=(d == ND - 1))
                        nc.tensor.matmul(ps1[:], lhs, wv_sb[:, d, EC:D],
                                         start=(d == 0), stop=(d == ND - 1))
                    nc.vector.tensor_copy(v[:, st, 0:EC], ps0[:])
                    nc.vector.tensor_copy(v[:, st, EC:D], ps1[:])

            # --- attention ---
            for qc in range(NQC):
                # scores^T for this q chunk: ET[kk, q] = exp(scale*XtT.T@AT)
                et_t = pet.tile([P, NS, QC], BF16, name="et_t", tag="et")
                for kk in range(NS):
                    st_ps = psST.tile([P, QC], F32, name="st_ps", tag="ps")
                    for e in range(ND):
                        nc.tensor.matmul(
                            st_ps[:],
                            kt[:, e, kk * P:(kk + 1) * P],
                            at[:, e, qc * QC:(qc + 1) * QC],
                            start=(e == 0), stop=(e == ND - 1))
                    nc.scalar.activation(et_t[:, kk, :], st_ps[:], AF.Exp,
                                         scale=SCALE)

                # attended[q_t, :] = (ET.T @ [V|1]) * recip + bv
                # three chains per q_tile: cols 0:512, 512:768, 768:1025
                # (last includes the ones column => softmax denominators).
                # Every chain streams >=256 rows, so each matmul's
                # ldweights hides under the previous stream.
                C1, C2 = 512, 768
                for qs in range(NH):
                    q_tile = qc * NH + qs
                    last = q_tile == NS - 1
                    recip = pstat.tile([P, 1], F32, name="recip", tag="recip")
                    ob = pout.tile([P, D], F32, name="ob", tag="ob")
                    if not last:
                        a0 = psAV.tile([P, EC], F32, name="a0", tag="ps")
                        a1 = psAV.tile([P, EC], F32, name="a1", tag="ps")
                        a2 = psAV.tile([P, EC], F32, name="a2", tag="ps")
                        for kk in range(NS):
                            lhs = et_t[:, kk, qs * P:(qs + 1) * P]
                            nc.tensor.matmul(a0[:], lhs, v[:, kk, 0:C1],
                                             start=(kk == 0),
                                             stop=(kk == NS - 1))
                            nc.tensor.matmul(a1[:, 0:C2 - C1], lhs,
                                             v[:, kk, C1:C2],
                                             start=(kk == 0),
                                             stop=(kk == NS - 1))
                            nc.tensor.matmul(a2[:, 0:D + 1 - C2], lhs,
                                             v[:, kk, C2:D + 1],
                                             start=(kk == 0),
                                             stop=(kk == NS - 1))
                        nc.vector.reciprocal(recip[:], a2[:, D - C2:D - C2 + 1])
                        nc.vector.scalar_tensor_tensor(
                            ob[:, 0:C1], a0[:], recip[:], bv_bcast[:, 0:C1],
                            op0=ALU.mult, op1=ALU.add)
                        nc.vector.scalar_tensor_tensor(
                            ob[:, C1:C2], a1[:, 0:C2 - C1], recip[:],
                            bv_bcast[:, C1:C2],
                            op0=ALU.mult, op1=ALU.add)
                        nc.vector.scalar_tensor_tensor(
                            ob[:, C2:D], a2[:, 0:D - C2], recip[:],
                            bv_bcast[:, C2:D],
                            op0=ALU.mult, op1=ALU.add)
                        eng = nc.sync if q_tile % 2 == 0 else nc.scalar
                        eng.dma_start(
                            out_d[q_tile * P:(q_tile + 1) * P, :], ob[:])
                    else:
                        # final tile: denominator chain finishes in the
                        # first pass (interleaved with cols 0:512), so
                        # the normalize+store of cols 0:512 and 768:1024
                        # overlaps the lone trailing 512:768 chain — the
                        # tail after the kernel's last matmul is a single
                        # 256-col drain + store.
                        H1, H2, H3 = 256, 512, 768
                        c0 = psAV.tile([P, H1], F32, name="c0", tag="ps")
                        c1 = psAV.tile([P, H2 - H1], F32, name="c1", tag="ps")
                        cd = psAV.tile([P, D + 1 - H3], F32, name="cd",
                                       tag="ps")
                        c2 = psAV.tile([P, H3 - H2], F32, name="c2", tag="ps")
                        for kk in range(NS):
                            lhs = et_t[:, kk, qs * P:(qs + 1) * P]
                            nc.tensor.matmul(cd[:], lhs, v[:, kk, H3:D + 1],
                                             start=(kk == 0),
                                             stop=(kk == NS - 1))
                            nc.tensor.matmul(c0[:], lhs, v[:, kk, 0:H1],
                                             start=(kk == 0),
                                             stop=(kk == NS - 1))
                            nc.tensor.matmul(c1[:], lhs, v[:, kk, H1:H2],
                                             start=(kk == 0),
                                             stop=(kk == NS - 1))
                        nc.vector.reciprocal(recip[:], cd[:, D - H3:D - H3 + 1])
                        nc.vector.scalar_tensor_tensor(
                            ob[:, H3:D], cd[:, 0:D - H3], recip[:],
                            bv_bcast[:, H3:D],
                            op0=ALU.mult, op1=ALU.add)
                        nc.scalar.dma_start(
                            out_d[q_tile * P:(q_tile + 1) * P, H3:D],
                            ob[:, H3:D])
                        nc.vector.scalar_tensor_tensor(
                            ob[:, 0:H1], c0[:], recip[:], bv_bcast[:, 0:H1],
                            op0=ALU.mult, op1=ALU.add)
                        nc.sync.dma_start(
                            out_d[q_tile * P:(q_tile + 1) * P, 0:H1],
                            ob[:, 0:H1])
                        nc.vector.scalar_tensor_tensor(
                            ob[:, H1:H2], c1[:], recip[:], bv_bcast[:, H1:H2],
                            op0=ALU.mult, op1=ALU.add)
                        nc.scalar.dma_start(
                            out_d[q_tile * P:(q_tile + 1) * P, H1:H2],
                            ob[:, H1:H2])
                        for kk in range(NS):
                            lhs = et_t[:, kk, qs * P:(qs + 1) * P]
                            nc.tensor.matmul(c2[:], lhs, v[:, kk, H2:H3],
                                             start=(kk == 0),
                                             stop=(kk == NS - 1))
                        nc.vector.scalar_tensor_tensor(
                            ob[:, H2:H3], c2[:], recip[:], bv_bcast[:, H2:H3],
                            op0=ALU.mult, op1=ALU.add)
                        nc.sync.dma_start(
                            out_d[q_tile * P:(q_tile + 1) * P, H2:H3],
                            ob[:, H2:H3])


def get_nc():
    if "nc" not in _CACHE:
        _CACHE["nc"] = _build_nc()
    return _CACHE["nc"]


def _prep_inputs(image_emb, text_emb, Wq, bq, Wk, bk, Wv, bv):
    bf = ml_dtypes.bfloat16
    xi = np.asarray(image_emb)   # [B, S, D] f32
    xt = np.asarray(text_emb)
    wq = np.asarray(Wq, dtype=np.float32)
    wk = np.asarray(Wk, dtype=np.float32)

    # m host layout [P, ND(et), ND(d)*P]: m[p, et, d*P+c] = M[d*P+p, et*P+c]
    m = (wq.T @ wk).astype(bf)                       # [D, D]
    m = m.reshape(ND, P, ND, P).transpose(1, 2, 0, 3).reshape(P, ND, ND * P)
    m = np.ascontiguousarray(m)

    ca = np.asarray(bq, dtype=np.float32) @ wk       # [D]
    ca = np.ascontiguousarray(ca.reshape(ND, P).T)   # [P, ND]

    # wvt [P, ND(d), D(e)]: wvt[p, d, e] = Wv[e, d*P+p]
    wvt = np.asarray(Wv).T.astype(bf).reshape(ND, P, D).transpose(1, 0, 2)
    wvt = np.ascontiguousarray(wvt)

    # xt [B, P, ND(d), S]: XtT tile order
    xtT = xt.transpose(0, 2, 1).astype(bf)           # [B, D, S]
    xtr = np.ascontiguousarray(
        xtT.reshape(B, ND, P, S).transpose(0, 2, 1, 3))

    # xi [B, P, NQC, ND*QC]: xi[b, p, qc, d*QC+c] = XiT[b, d*P+p, qc*QC+c]
    xiT = xi.transpose(0, 2, 1).astype(bf)           # [B, D, S]
    xir = np.ascontiguousarray(
        xiT.reshape(B, ND, P, NQC, QC).transpose(0, 2, 3, 1, 4)
        .reshape(B, P, NQC, ND * QC))

    bv2 = np.ascontiguousarray(
        np.broadcast_to(np.asarray(bv, dtype=np.float32), (P, D)))
    in_maps = []
    for b in range(B):
        in_maps.append({
            "xi": xir[b], "xt": xtr[b],
            "m": m, "wvt": wvt, "ca": ca, "bv2": bv2,
        })
    return in_maps


def run(image_emb, text_emb, Wq, bq, Wk, bk, Wv, bv, trace=False, **spmd_kwargs):
    nc = get_nc()
    in_maps = _prep_inputs(image_emb, text_emb, Wq, bq, Wk, bk, Wv, bv)
    res = run_bass_kernel_spmd(nc, in_maps, list(range(B)), trace=trace,
                               **spmd_kwargs)
    out = np.stack([res.results[b]["out"] for b in range(B)], axis=0)
    return out, res


def kernel(image_emb, text_emb, edge_index=None, Wq=None, bq=None, Wk=None,
           bk=None, Wv=None, bv=None, **_unused):
    out, _ = run(image_emb, text_emb, Wq, bq, Wk, bk, Wv, bv, trace=False)
    return out


# revision 11
# speedup vs baseline: 1.0024x; 1.0024x over previous
"""Trainium2 Bass kernel for single-head cross-modal attention.

Problem: B=8, S=2048, D=1024 (fp32 inputs)
    q = image_emb @ Wq.T + bq
    k = text_emb  @ Wk.T + bk
    v = text_emb  @ Wv.T + bv
    out = softmax(q @ k.T / sqrt(D)) @ v
Sharding: data-parallel over batch — core b handles batch element b.

Key algebraic restructure (kills the on-device K projection):
    q k^T = Xi (Wq^T Wk) Xt^T + (per-query-row constants) + 1 (bq^T Wk) Xt^T
Softmax is row-shift invariant, so the row-constant terms drop. With
host-precomputed M = Wq^T Wk and c = bq @ Wk:
    scores ~ (Xi M + 1 c^T) Xt^T
The Q projection becomes A = Xi M + c (same cost, bias reused) and K^T
is just Xt^T — DMA'd once into SBUF and reused both as the scores
stationary and as the V-projection stationary.

Startup (trace-driven): nothing can land in SBUF before ~12.5us (code
load ~3us, engine barrier, first DMA issue ~7.2us, DMA pipe latency).
Meanwhile the PE HAM clock gate is cold (1.2 GHz) and only goes warm
after ~3.4us of gap-free matmul activity. So the kernel front-loads a
dependency-free bf16 warm-up chain (memset tile, accumulated into a
dead PSUM bank) sized to cover the DMA latency window: by the time the
first A-phase operand arrives, the PE is already at 2.4 GHz and the A
phase streams gap-free. The bv broadcast is host-prepared and DMA'd
(the old on-chip fp32 ones@bv matmul broadcast ran in 2-pass fp32 mode
and wasted ~4us of the warm-up window).

DMA strategy: every dma_start costs ~0.7us on the issuing sequencer and
the two HWDGE rings share the 16 SDMA engines (~0.43 MB/us aggregate,
global FIFO-ish in issue order). Descriptors are ordered by first use:
first A-chain operands (m row 0, xi chunk 0) lead both rings, the rest
of m/xi follow interleaved, V/attention bulk (wv, kt) and the
bv broadcast ride the tails.

Per-core dataflow (matmuls bf16, fp32 PSUM):
  - AT[d',q] per 512-col chunk: stationary M[d,d'_tile], moving XiT.
  - scoresT[k,q] = kt_tile.T @ AT -> exp -> stationary of P@V: the
    2048x2048 probability matrix is never transposed on chip.
  - no-max softmax (scores ~ N(0,1)); V carries an appended ones
    column, and P@V runs as three chains (512/256/257 cols) so the
    softmax denominators fall out of the third chain's last column and
    every matmul's ldweights hides under a >=256-row stream; final
    normalize fused with the bv add in one DVE op per chunk.
  - last q_tile runs denominator-chain-first (257+256+256 interleaved,
    then a lone 256-col chain) so only one small drain+store trails the
    final matmul.
  - all pools live in one scope (no mid-kernel scope-exit barrier);
    single 8-bank PSUM ring shared by every phase.
"""

import sys
import os

for _p in ("/opt/trn_rl_repo", "/root/.axon_site/_ro/trn_rl_repo"):
    if os.path.isdir(_p) and _p not in sys.path:
        sys.path.insert(0, _p)

import numpy as np
import ml_dtypes

import concourse.bass as bass
import concourse.mybir as mybir
import concourse.tile as tile
from concourse import bacc
from concourse.bass_utils import run_bass_kernel_spmd

BF16 = mybir.dt.bfloat16
F32 = mybir.dt.float32
AF = mybir.ActivationFunctionType
ALU = mybir.AluOpType

B, S, D = 8, 2048, 1024
P = 128
ND = D // P          # 8  d tiles
NS = S // P          # 16 s tiles
QC = 512             # q chunk width (matmul free dim / PSUM bank)
NQC = S // QC        # 4
EC = 512             # e chunk width for V / output
SCALE = 1.0 / float(np.sqrt(D))
NWARM = 20           # dummy warm-up matmuls; sized so the chain ends
                     # ~14.6us — past the HAM warm point and within
                     # ~0.7us of a slow-ramp first-descriptor arrival,
                     # so supply jitter can't open a re-throttling gap

_CACHE = {}


def _build_nc():
    nc = bacc.Bacc("TRN2", target_bir_lowering=False, debug=False, num_devices=8)

    # all bulk tensors are pre-relayouted on host to SBUF tile order
    xi_d = nc.dram_tensor("xi", [P, NQC, ND * QC], BF16, kind="ExternalInput").ap()
    xt_d = nc.dram_tensor("xt", [P, ND, S], BF16, kind="ExternalInput").ap()
    m_d = nc.dram_tensor("m", [P, ND, ND * P], BF16, kind="ExternalInput").ap()
    wvt_d = nc.dram_tensor("wvt", [P, ND, D], BF16, kind="ExternalInput").ap()
    ca_d = nc.dram_tensor("ca", [P, ND], F32, kind="ExternalInput").ap()
    bv2_d = nc.dram_tensor("bv2", [P, D], F32, kind="ExternalInput").ap()
    out_d = nc.dram_tensor("out", [S, D], F32, kind="ExternalOutput").ap()

    with tile.TileContext(nc) as tc:
        _emit(nc, tc, xi_d, xt_d, m_d, wvt_d, ca_d, bv2_d, out_d)
    nc.compile()
    return nc


def _emit(nc, tc, xi_d, xt_d, m_d, wvt_d, ca_d, bv2_d, out_d):
    NH = QC // P  # 4 q_tiles per chunk
    with (
        tc.tile_pool(name="const", bufs=1) as pc,
        tc.tile_pool(name="qkv", bufs=1) as pqkv,
    ):
        # persistent activations
        at = pqkv.tile([P, ND, S], BF16, name="at", tag="at")    # AT[d',q]
        kt = pqkv.tile([P, ND, S], BF16, name="kt", tag="kt")    # XtT[d',k]
        # V[s, e] with a ones column appended at e=D: the P@V row sums
        # (softmax denominators) fall out of the last PV chain, so no
        # 1-column rowsum matmuls (whose ldweights never hide) are needed.
        v = pqkv.tile([P, NS, D + 1], BF16, name="v", tag="v")

        # constants
        bias_a = pc.tile([P, ND], F32, name="bias_a", tag="bias_a")
        bv_bcast = pc.tile([P, D], F32, name="bv_bcast", tag="bv_bcast")
        warm = pc.tile([P, QC], BF16, name="warm", tag="warm")

        with (
            tc.tile_pool(name="w", bufs=1) as pw,
            tc.tile_pool(name="xs", bufs=1) as pxs,
            tc.tile_pool(name="psP", bufs=8, space="PSUM") as psP,
            tc.tile_pool(name="et", bufs=2) as pet,
            tc.tile_pool(name="outp", bufs=1) as pout,
            tc.tile_pool(name="stat", bufs=4) as pstat,
        ):
            psST = psAV = psP
            # m_sb[:, et, d*P:(d+1)*P] = M[d-block, et-block] (et-major!)
            m_sb = pw.tile([P, ND, ND * P], BF16, name="m_sb", tag="m_sb")
            wv_sb = pw.tile([P, ND, D], BF16, name="wv_sb", tag="wv_sb")
            # xc[:, qc, d*QC:(d+1)*QC] = XiT[d-block, qc-chunk]
            xc = pxs.tile([P, NQC, ND * QC], BF16, name="xc", tag="xs")

            # --- bulk DMAs, ordered by first use. Two hard constraints
            # (trace-measured):
            #  * the rings share ONE pool of 8 completion semaphores,
            #    allocated in global dma_start program order; descriptor
            #    9+ REUSES a semaphore and its issue blocks until the
            #    previous user completes. So the 8 A-phase-critical
            #    descriptors go first, and each reuser is paired with an
            #    early-completing predecessor.
            #  * aggregate delivery is ~0.4 MB/us from ~9.5us, roughly
            #    in issue order; a warm A chain consumes 0.22 MB/us.
            # NOTE: gpsimd triggers lower to qPoolDynamic (slow SW
            # completions) — avoid; everything rides sync/scalar HWDGE.
            # Delivery is ~0.4 MB/us aggregate in issue order, and is
            # packet-rate-bound: m descriptors spanning >=2 et-rows move
            # as 4KB packets (per-partition-contiguous), single rows
            # only 2KB — so m ships as 2/2/4-row chunks. First A-chain
            # bytes lead; everything is just-in-time for a fully warm,
            # gap-free A phase from ~12.4us.
            # (xi/m slices below keep >=4KB per-partition-contiguous
            # runs; finer slicing halves the packet rate and loses more
            # than the granularity wins.)
            nc.sync.dma_start(m_sb[:, 0:2, :], m_d[:, 0:2, :])    # 512KB
            nc.scalar.dma_start(xc[:, 0, 0:4 * QC], xi_d[:, 0, 0:4 * QC])
            nc.sync.dma_start(xc[:, 0, 4 * QC:], xi_d[:, 0, 4 * QC:])
            nc.scalar.dma_start(m_sb[:, 2:4, :], m_d[:, 2:4, :])  # 512KB
            nc.sync.dma_start(m_sb[:, 4:8, :], m_d[:, 4:8, :])    # 1MB
            nc.scalar.dma_start(bias_a[:], ca_d[:])               # 4KB
            nc.sync.dma_start(xc[:, 1, :], xi_d[:, 1, :])         # 1MB
            nc.scalar.dma_start(xc[:, 2, :], xi_d[:, 2, :])
            # -- semaphore pool (8) exhausted: each of the following
            # reuses the semaphore of the descriptor 8 positions earlier
            # and issues only once that one completes — paired so every
            # issue-wait is satisfied before the data is needed.
            nc.sync.dma_start(xc[:, 3, :], xi_d[:, 3, :])
            nc.scalar.dma_start(wv_sb[:], wvt_d[:])               # 2MB
            nc.sync.dma_start(kt[:, 0:4, :], xt_d[:, 0:4, :])
            nc.scalar.dma_start(kt[:, 4:8, :], xt_d[:, 4:8, :])
            # bv broadcast: first consumer is the PV phase (~165us)
            nc.sync.dma_start(bv_bcast[:], bv2_d[:])              # 512KB
            nc.vector.memset(warm[:], 1.0)
            nc.vector.memset(v[:, :, D:D + 1], 1.0)

            # --- dependency-free warm-up chain: keeps the PE busy from
            # ~7.5us (right after the framework preamble) through the
            # ~12.5us DMA pipe latency, flipping the HAM clock gate to
            # 2.4 GHz before the first real chain. Accumulates ones into
            # a dead PSUM bank; never read.
            ps_w = psP.tile([P, QC], F32, name="ps_w", tag="ps")
            for i in range(NWARM):
                nc.tensor.matmul(ps_w[:], warm[:, 0:P], warm[:],
                                 start=(i == 0), stop=(i == NWARM - 1))

            # --- AT[d'_t, qc] = sum_d M[d, d'_t].T @ XiT[d, qc]  (+ c) ---
            # qc 0 runs during the DMA supply transient: its first four
            # et chains are emitted as interleaved d-half blocks, so all
            # work needing only the earliest descriptors (m rows 0-1,
            # xi chunk 0 first half) runs first and a late xc0b/m23
            # stalls the PE briefly instead of ~3us (which would also
            # re-throttle the HAM clock gate).
            ps_h = [psP.tile([P, QC], F32, name=f"psh{et}", tag="ps")
                    for et in range(4)]
            for half in range(2):
                for ep in range(2):
                    for d in range(half * 4, half * 4 + 4):
                        for et in (2 * ep, 2 * ep + 1):
                            nc.tensor.matmul(
                                ps_h[et][:], m_sb[:, et, d * P:(d + 1) * P],
                                xc[:, 0, d * QC:(d + 1) * QC],
                                start=(d == 0), stop=(d == ND - 1))
            for et in range(4):
                nc.vector.tensor_scalar_add(
                    at[:, et, 0:QC], ps_h[et][:], bias_a[:, et:et + 1])
            for qc in range(NQC):
                for et in range(4 if qc == 0 else 0, ND):
                    ps = psP.tile([P, QC], F32, name="ps", tag="ps")
                    for d in range(ND):
                        nc.tensor.matmul(
                            ps[:], m_sb[:, et, d * P:(d + 1) * P],
                            xc[:, qc, d * QC:(d + 1) * QC],
                            start=(d == 0), stop=(d == ND - 1))
                    nc.vector.tensor_scalar_add(
                        at[:, et, qc * QC:(qc + 1) * QC], ps[:],
                        bias_a[:, et:et + 1])

            # --- V[s_t, e] = sum_d XtT[d, s_t].T @ Wv^T[d, e] ---
            for vc in range(NQC):
                for si in range(NH):
                    st = vc * NH + si
                    ps0 = psP.tile([P, EC], F32, name="ps0", tag="ps")
                    ps1 = psP.tile([P, EC], F32, name="ps1", tag="ps")
                    for d in range(ND):
                        lhs = kt[:, d, st * P:(st + 1) * P]
                        nc.tensor.matmul(ps0[:], lhs, wv_sb[:, d, 0:EC],
                                         start=(d == 0), stop=(d == ND - 1))
                        nc.tensor.matmul(ps1[:], lhs, wv_sb[:, d, EC:D],
                                         start=(d == 0), stop=(d == ND - 1))
                    nc.vector.tensor_copy(v[:, st, 0:EC], ps0[:])
                    nc.vector.tensor_copy(v[:, st, EC:D], ps1[:])

            # --- attention ---
            for qc in range(NQC):
                # scores^T for this q chunk: ET[kk, q] = exp(scale*XtT.T@AT)
                et_t = pet.tile([P, NS, QC], BF16, name="et_t", tag="et")
                for kk in range(NS):
                    st_ps = psST.tile([P, QC], F32, name="st_ps", tag="ps")
                    for e in range(ND):
                        nc.tensor.matmul(
                            st_ps[:],
                            kt[:, e, kk * P:(kk + 1) * P],
                            at[:, e, qc * QC:(qc + 1) * QC],
                            start=(e == 0), stop=(e == ND - 1))
                    nc.scalar.activation(et_t[:, kk, :], st_ps[:], AF.Exp,
                                         scale=SCALE)

                # attended[q_t, :] = (ET.T @ [V|1]) * recip + bv
                # three chains per q_tile: cols 0:512, 512:768, 768:1025
                # (last includes the ones column => softmax denominators).
                # Every chain streams >=256 rows, so each matmul's
                # ldweights hides under the previous stream.
                C1, C2 = 512, 768
                for qs in range(NH):
                    q_tile = qc * NH + qs
                    last = q_tile == NS - 1
                    recip = pstat.tile([P, 1], F32, name="recip", tag="recip")
                    ob = pout.tile([P, D], F32, name="ob", tag="ob")
                    if not last:
                        a0 = psAV.tile([P, EC], F32, name="a0", tag="ps")
                        a1 = psAV.tile([P, EC], F32, name="a1", tag="ps")
                        a2 = psAV.tile([P, EC], F32, name="a2", tag="ps")
                        for kk in range(NS):
                            lhs = et_t[:, kk, qs * P:(qs + 1) * P]
                            nc.tensor.matmul(a0[:], lhs, v[:, kk, 0:C1],
                                             start=(kk == 0),
                                             stop=(kk == NS - 1))
                            nc.tensor.matmul(a1[:, 0:C2 - C1], lhs,
                                             v[:, kk, C1:C2],
                                             start=(kk == 0),
                                             stop=(kk == NS - 1))
                            nc.tensor.matmul(a2[:, 0:D + 1 - C2], lhs,
                                             v[:, kk, C2:D + 1],
                                             start=(kk == 0),
                                             stop=(kk == NS - 1))
                        nc.vector.reciprocal(recip[:], a2[:, D - C2:D - C2 + 1])
                        nc.vector.scalar_tensor_tensor(
                            ob[:, 0:C1], a0[:], recip[:], bv_bcast[:, 0:C1],
                            op0=ALU.mult, op1=ALU.add)
                        nc.vector.scalar_tensor_tensor(
                            ob[:, C1:C2], a1[:, 0:C2 - C1], recip[:],
                            bv_bcast[:, C1:C2],
                            op0=ALU.mult, op1=ALU.add)
                        nc.vector.scalar_tensor_tensor(
                            ob[:, C2:D], a2[:, 0:D - C2], recip[:],
                            bv_bcast[:, C2:D],
                            op0=ALU.mult, op1=ALU.add)
                        eng = nc.sync if q_tile % 2 == 0 else nc.scalar
                        eng.dma_start(
                            out_d[q_tile * P:(q_tile + 1) * P, :], ob[:])
                    else:
                        # final tile: denominator chain finishes in the
                        # first pass (interleaved with cols 0:512), so
                        # the normalize+store of cols 0:512 and 768:1024
                        # overlaps the lone trailing 512:768 chain — the
                        # tail after the kernel's last matmul is a single
                        # 256-col drain + store.
                        H1, H2, H3 = 256, 512, 768
                        c0 = psAV.tile([P, H1], F32, name="c0", tag="ps")
                        c1 = psAV.tile([P, H2 - H1], F32, name="c1", tag="ps")
                        cd = psAV.tile([P, D + 1 - H3], F32, name="cd",
                                       tag="ps")
                        c2 = psAV.tile([P, H3 - H2], F32, name="c2", tag="ps")
                        for kk in range(NS):
                            lhs = et_t[:, kk, qs * P:(qs + 1) * P]
                            nc.tensor.matmul(cd[:], lhs, v[:, kk, H3:D + 1],
                                             start=(kk == 0),
                                             stop=(kk == NS - 1))
                            nc.tensor.matmul(c0[:], lhs, v[:, kk, 0:H1],
                                             start=(kk == 0),
                                             stop=(kk == NS - 1))
                            nc.tensor.matmul(c1[:], lhs, v[:, kk, H1:H2],
                                             start=(kk == 0),
                                             stop=(kk == NS - 1))
                        nc.vector.reciprocal(recip[:], cd[:, D - H3:D - H3 + 1])
                        nc.vector.scalar_tensor_tensor(
                            ob[:, H3:D], cd[:, 0:D - H3], recip[:],
                            bv_bcast[:, H3:D],
                            op0=ALU.mult, op1=ALU.add)
                        nc.scalar.dma_start(
                            out_d[q_tile * P:(q_tile + 1) * P, H3:D],
                            ob[:, H3:D])
                        nc.vector.scalar_tensor_tensor(
                            ob[:, 0:H1], c0[:], recip[:], bv_bcast[:, 0:H1],
                            op0=ALU.mult, op1=ALU.add)
                        nc.sync.dma_start(
                            out_d[q_tile * P:(q_tile + 1) * P, 0:H1],
                            ob[:, 0:H1])
                        nc.vector.scalar_tensor_tensor(
                            ob[:, H1:H2], c1[:], recip[:], bv_bcast[:, H1:H2],
                            op0=ALU.mult, op1=ALU.add)
                        nc.scalar.dma_start(
                            out_d[q_tile * P:(q_tile + 1) * P, H1:H2],
                            ob[:, H1:H2])
                        for kk in range(NS):
                            lhs = et_t[:, kk, qs * P:(qs + 1) * P]
                            nc.tensor.matmul(c2[:], lhs, v[:, kk, H2:H3],
                                             start=(kk == 0),
                                             stop=(kk == NS - 1))
                        nc.vector.scalar_tensor_tensor(
                            ob[:, H2:H3], c2[:], recip[:], bv_bcast[:, H2:H3],
                            op0=ALU.mult, op1=ALU.add)
                        nc.sync.dma_start(
                            out_d[q_tile * P:(q_tile + 1) * P, H2:H3],
                            ob[:, H2:H3])


def get_nc():
    if "nc" not in _CACHE:
        _CACHE["nc"] = _build_nc()
    return _CACHE["nc"]


def _prep_inputs(image_emb, text_emb, Wq, bq, Wk, bk, Wv, bv):
    bf = ml_dtypes.bfloat16
    xi = np.asarray(image_emb)   # [B, S, D] f32
    xt = np.asarray(text_emb)
    wq = np.asarray(Wq, dtype=np.float32)
    wk = np.asarray(Wk, dtype=np.float32)

    # m host layout [P, ND(et), ND(d)*P]: m[p, et, d*P+c] = M[d*P+p, et*P+c]
    m = (wq.T @ wk).astype(bf)                       # [D, D]
    m = m.reshape(ND, P, ND, P).transpose(1, 2, 0, 3).reshape(P, ND, ND * P)
    m = np.ascontiguousarray(m)

    ca = np.asarray(bq, dtype=np.float32) @ wk       # [D]
    ca = np.ascontiguousarray(ca.reshape(ND, P).T)   # [P, ND]

    # wvt [P, ND(d), D(e)]: wvt[p, d, e] = Wv[e, d*P+p]
    wvt = np.asarray(Wv).T.astype(bf).reshape(ND, P, D).transpose(1, 0, 2)
    wvt = np.ascontiguousarray(wvt)

    # xt [B, P, ND(d), S]: XtT tile order
    xtT = xt.transpose(0, 2, 1).astype(bf)           # [B, D, S]
    xtr = np.ascontiguousarray(
        xtT.reshape(B, ND, P, S).transpose(0, 2, 1, 3))

    # xi [B, P, NQC, ND*QC]: xi[b, p, qc, d*QC+c] = XiT[b, d*P+p, qc*QC+c]
    xiT = xi.transpose(0, 2, 1).astype(bf)           # [B, D, S]
    xir = np.ascontiguousarray(
        xiT.reshape(B, ND, P, NQC, QC).transpose(0, 2, 3, 1, 4)
        .reshape(B, P, NQC, ND * QC))

    bv2 = np.ascontiguousarray(
        np.broadcast_to(np.asarray(bv, dtype=np.float32), (P, D)))
    in_maps = []
    for b in range(B):
        in_maps.append({
            "xi": xir[b], "xt": xtr[b],
            "m": m, "wvt": wvt, "ca": ca, "bv2": bv2,
        })
    return in_maps


def run(image_emb, text_emb, Wq, bq, Wk, bk, Wv, bv, trace=False, **spmd_kwargs):
    nc = get_nc()
    in_maps = _prep_inputs(image_emb, text_emb, Wq, bq, Wk, bk, Wv, bv)
    res = run_bass_kernel_spmd(nc, in_maps, list(range(B)), trace=trace,
                               **spmd_kwargs)
    out = np.stack([res.results[b]["out"] for b in range(B)], axis=0)
    return out, res


def kernel(image_emb, text_emb, edge_index=None, Wq=None, bq=None, Wk=None,
           bk=None, Wv=None, bv=None, **_unused):
    out, _ = run(image_emb, text_emb, Wq, bq, Wk, bk, Wv, bv, trace=False)
    return out
